# revision 18
# baseline (speedup 1.0000x reference)
"""Trainium2 Bass kernel for nn_CategoryBranch2 (3 conv blocks + 2 BiGRU layers).

Distribution: data-parallel over batch B=8 -> one sample per NeuronCore; each
core runs the whole pipeline for its sample.

Device plan per core:
  phase1  conv blocks, T-tiled (8 tiles of 128 steps, halo recompute).
          conv1 via im2col over 9 shift-partitions (single K=128 matmul per
          output chunk); conv2/3 via 9 shift-accumulated matmuls. Epilogues
          fuse bias+relu (ACT), maxpool over f (DVE max), BN affine (DVE).
          conv3 output is written in the permuted layout d' = f*256 + c so the
          GRU input matmul needs no device transpose (wi1 columns are permuted
          on host to match); staged to DRAM as yp[128, 32, 1024].
  phase2  gx1[dir] = wi1' @ y' + (bi + bh_rz), fp32, SBUF-resident,
          TIME-MAJOR [128, 1024, 12] per dir; bwd stored time-reversed.
  phase3  L1 GRU scans (1024 steps). Per step: one 16-col identity-matmul
          preload per dir injects gx_rz/gx_n/bh_n (pre-replicated into the
          gx rows) into PSUM, then 96 bf16 weight matmuls, then ONE lockstep
          tail for both dirs: sigmoid -> q=r*v -> npre=q+gxn -> tanh(n) ->
          f-scan (tensor_tensor_scan FMA: zc*n+z*h via (0,x) interleave) ->
          tanh(h).  zc/a are computed on GPSIMD off the critical path.
  phase4  gx2 + L2 scans (512 steps), same machinery; final sum ->
          out[128, 4, 256] (host reassembles).

PSUM per-step layout (ONE bank tile [128,32] for both dirs, 4 buffers;
base=16d per dir): u 0:8 | gxn 8:12 | v 12:16.  The gx rows are 16 wide:
[rz(8) | n(4) | bhn replica(4)] so a single contiguous identity matmul
preloads everything (start=True on d0 zeroes the bank).
SBUF T [128,32]: r at 8d+2j+1, z->1-z in place at 16+8d+2j+1; even cols
stay 0 (memset once) = multiply-by-zero state resets for the f-scan.

Self-contained: hardcodes all shapes; host does only numpy weight re-layouts.
"""

import numpy as np
import ml_dtypes

import concourse.bacc as bacc
import concourse.bass as bass
import concourse.mybir as mybir
from concourse.tile import TileContext
from concourse.bass import ds
from concourse.bass_utils import run_bass_kernel_spmd

BF16 = ml_dtypes.bfloat16
E4M3 = ml_dtypes.float8_e4m3
F32 = mybir.dt.float32
BF = mybir.dt.bfloat16
FP8 = mybir.dt.float8e4
AF = mybir.ActivationFunctionType
OP = mybir.AluOpType
PE = mybir.EngineType.PE

BN_EPS = 1e-5

X9_LEN = 134 * 130          # 17420
X2_LEN = 132 * 66 + 66      # 8778
X3_LEN = 130 * 34 + 34      # 4454
YPST_LEN = 32 * 128         # 4096

_CACHED_NC = {}
import os as _os
NDUMMY = int(_os.environ.get('KNDUMMY', '0'))


def _scan_superstep(nc, ps, whs, gxvs, bhns, ident, Ts, npres, As, fouts, hs,
                    louts, tgs, s_out, dummies=None):
    """One time step, both dirs lockstep. PE: 1 contiguous preload matmul
    per dir (no h dep) + 96 weight matmuls; then one 6-hop tail shared by
    both dirs: sigma -> q -> npre -> tanh(n) -> f-scan -> tanh(h)."""
    # psum [128,32], base=16d per dir: u 0:8 | gxn 8:12 | v 12:16
    # T [128,32]: r at 8d+2j+1 (for q), z->zc in place at 16+8d+2j+1
    #             (f-scan d0-operand: zeros at evens = state resets)
    # A [128,16]: (n, a) pairs per k=4d+j.  h [128,8]: cols d*4+kc.
    T, npre, A, fout, h = Ts, npres, As, fouts, hs
    psum = ps.tile([128, 32], F32, tag="scan_psum", name="psum")
    # psum layout per dir (base=16d): u 0:8 | gxn 8:12 | v 12:16 -> preloads
    # are two contiguous identity matmuls per dir (no per-column stalls)
    for d in range(2):
        nc.tensor.matmul(psum[:, 16 * d:16 * d + 16], ident,
                         gxvs[d][:, ds(tgs[d], 1), 0:16],
                         start=(d == 0), stop=False, skip_group_check=True)
    for d in range(2):
        for kc in range(4):
            for mc in range(12):
                oc = 16 * d + mc if mc < 8 else 16 * d + 12 + (mc - 8)
                nc.tensor.matmul(
                    psum[:, oc:oc + 1],
                    whs[d][:, (kc * 12 + mc) * 128:(kc * 12 + mc + 1) * 128],
                    h[:, 4 * d + kc:4 * d + kc + 1],
                    start=False, stop=(kc == 3), skip_group_check=True)
    if dummies is not None:
        scrap, dpin, nd = dummies
        for k in range(nd):
            nc.tensor.matmul(scrap[0:1, :], dpin, whs[0][:, 0:512],
                             start=True, stop=True, skip_group_check=True)
    # sigma both dirs: u -> r at T[8d+2j+1], z at T[16+8d+2j+1]
    uin = psum[:].rearrange("p (d q g j) -> p q g d j", d=2, q=2, g=2)[:, 0]
    sout = T[:].rearrange("p (g d j e) -> p g d j e", g=2, d=2, e=2)[:, :, :, :, 1]
    nc.scalar.activation(sout, uin, AF.Sigmoid)
    # q = r*v ; npre = q + gxn  (plain DVE ops on contiguous psum views)
    pv = psum[:].rearrange("p (d c) -> p d c", d=2)
    rview = T[:, 0:16].rearrange("p (d j e) -> p d j e", d=2, e=2)[:, :, :, 1]
    q = npre[:, 0:8].rearrange("p (d j) -> p d j", d=2)
    nc.vector.tensor_tensor(q, rview, pv[:, :, 12:16], OP.mult)
    np2 = npre[:, 8:16].rearrange("p (d j) -> p d j", d=2)
    nc.vector.tensor_tensor(np2, q, pv[:, :, 8:12], OP.add)
    # tanh n -> A evens
    aview = A[:].rearrange("p (k e) -> p k e", e=2)
    nc.scalar.activation(aview[:, :, 0], npre[:, 8:16], AF.Tanh)
    # GPSIMD (off critical path): a = z*h -> A odds; zc = 1-z in place
    zview = T[:, 16:32].rearrange("p (k e) -> p k e", e=2)[:, :, 1]
    nc.gpsimd.tensor_tensor(aview[:, :, 1], zview, h[:], OP.mult)
    nc.gpsimd.tensor_scalar(zview, zview, -1.0, 1.0, OP.mult, OP.add)
    if dummies is not None:
        nc.gpsimd.tensor_copy(dummies[1], T[:, 1:2])
    # f = zc*n + a for both dirs in one scan op
    nc.vector.tensor_tensor_scan(fout[:], T[:, 16:32], A[:], 0.0,
                                 OP.mult, OP.add)
    # h' = tanh(f)
    fodd = fout[:].rearrange("p (k e) -> p k e", e=2)
    nc.scalar.activation(h[:], fodd[:, :, 1], AF.Tanh)
    if s_out is not None:
        for d in range(2):
            nc.vector.tensor_copy(louts[d][:, :, ds(s_out, 1)],
                                  h[:, 4 * d:4 * d + 4, None])


def _scan_blk(nc, ps, base, sbase, whs, gxvs, bhns, ident, Ts, npres, As,
              fouts, hs, louts, BLK, dummies=None):
    for pi in range(BLK // 2):
        for par in range(2):
            t_loc = pi * 2 + par
            tg = base + t_loc
            _scan_superstep(nc, ps, whs, gxvs, bhns, ident, Ts, npres, As,
                            fouts, hs, louts, (tg, tg),
                            (sbase + pi) if par == 0 else None, dummies)


def _scan_loop(nc, tc, ps, nblk, whs, gxvs, bhns, ident, Ts, npres, As, fouts,
               hs, louts, BLK=32, static=False, dummies=None):
    if nblk == 0:
        return
    if static:
        for blki in range(nblk):
            _scan_blk(nc, ps, blki * BLK, blki * (BLK // 2), whs, gxvs, bhns,
                      ident, Ts, npres, As, fouts, hs, louts, BLK, dummies)
        return
    with tc.For_i(0, nblk, 1, hint_engines=(PE,)) as blk:
        _scan_blk(nc, ps, blk * BLK, blk * (BLK // 2), whs, gxvs, bhns,
                  ident, Ts, npres, As, fouts, hs, louts, BLK, dummies)


def build_nc(debug_outputs=False, nblk1=16, nblk2=8, static=False, ndummy=0):
    nc = bacc.Bacc("TRN2", target_bir_lowering=False, debug=False, num_devices=8)

    # ---------------- inputs ----------------
    xp_d = nc.dram_tensor("xp", [1031 * 130], BF, kind="ExternalInput")
    ident_d = nc.dram_tensor("ident", [128, 128], BF, kind="ExternalInput")
    w1s_d = nc.dram_tensor("w1s", [128, 64], BF, kind="ExternalInput")
    w2s_d = nc.dram_tensor("w2s", [9, 128, 128], BF, kind="ExternalInput")
    w3s_d = nc.dram_tensor("w3s", [9, 128, 2, 128], BF, kind="ExternalInput")
    cb1_d = nc.dram_tensor("cb1", [64, 1], F32, kind="ExternalInput")
    sc1_d = nc.dram_tensor("sc1", [64, 1], F32, kind="ExternalInput")
    sh1_d = nc.dram_tensor("sh1", [64, 1], F32, kind="ExternalInput")
    cb2_d = nc.dram_tensor("cb2", [128, 1], F32, kind="ExternalInput")
    sc2_d = nc.dram_tensor("sc2", [128, 1], F32, kind="ExternalInput")
    sh2_d = nc.dram_tensor("sh2", [128, 1], F32, kind="ExternalInput")
    cb3_d = nc.dram_tensor("cb3", [128, 2], F32, kind="ExternalInput")
    sc3_d = nc.dram_tensor("sc3", [128, 2], F32, kind="ExternalInput")
    sh3_d = nc.dram_tensor("sh3", [128, 2], F32, kind="ExternalInput")
    wi1_d = nc.dram_tensor("wi1", [2, 12, 128, 32, 128], BF, kind="ExternalInput")
    gxb1_d = nc.dram_tensor("gxb1", [2, 128, 12], F32, kind="ExternalInput")
    wh1_d = nc.dram_tensor("wh1", [2, 128, 4 * 12 * 128], BF, kind="ExternalInput")
    bhn1_d = nc.dram_tensor("bhn1", [2, 128, 4], BF, kind="ExternalInput")
    wi2_d = nc.dram_tensor("wi2", [2, 128, 12 * 4 * 128], BF, kind="ExternalInput")
    gxb2_d = nc.dram_tensor("gxb2", [2, 128, 12], F32, kind="ExternalInput")
    wh2_d = nc.dram_tensor("wh2", [2, 128, 4 * 12 * 128], BF, kind="ExternalInput")
    bhn2_d = nc.dram_tensor("bhn2", [2, 128, 4], BF, kind="ExternalInput")

    out_d = nc.dram_tensor("out", [128, 4, 256], F32, kind="ExternalOutput")
    yp_kind = "ExternalOutput" if debug_outputs else "Internal"
    yp_d = nc.dram_tensor("yp", [128, 32, 1024], BF, kind=yp_kind)
    if debug_outputs:
        gx_dbg = nc.dram_tensor("gx_dbg", [2, 128, 1024, 12], BF,
                                kind="ExternalOutput")
        l2in_dbg = nc.dram_tensor("l2in_dbg", [128, 4 * 512], F32,
                                  kind="ExternalOutput")

    with TileContext(nc) as tc:
      with tc.tile_pool(name="keep", bufs=1) as keep, \
           tc.tile_pool(name="arena1", bufs=1) as ar1, \
           tc.tile_pool(name="arena2", bufs=1) as ar2, \
           tc.tile_pool(name="scan_ps", bufs=4, space="PSUM") as ps, \
           tc.tile_pool(name="warm_ps", bufs=1, space="PSUM") as wps:
        # ---- long-lived small tiles
        ident = keep.tile([128, 128], BF, tag="ident")
        nc.sync.dma_start(ident, ident_d[:])
        gxb1_sb = []
        bhn1_sb = []
        louts1 = []
        gxb2_sb = []
        bhn2_sb = []
        louts2 = []
        for d in range(2):
            t = keep.tile([128, 12], F32, tag=f"gxb1_{d}")
            nc.sync.dma_start(t, gxb1_d[d])
            gxb1_sb.append(t)
            t = keep.tile([128, 4], BF, tag=f"bhn1_{d}")
            nc.sync.dma_start(t, bhn1_d[d])
            bhn1_sb.append(t)
            lo = keep.tile([128, 4, 512], BF, tag=f"lo1_{d}", name=f"lo1_{d}")
            nc.vector.memset(lo[:], 0.0)
            louts1.append(lo)
            t = keep.tile([128, 12], F32, tag=f"gxb2_{d}")
            nc.sync.dma_start(t, gxb2_d[d])
            gxb2_sb.append(t)
            t = keep.tile([128, 4], BF, tag=f"bhn2_{d}")
            nc.sync.dma_start(t, bhn2_d[d])
            bhn2_sb.append(t)
            lo2 = keep.tile([128, 4, 256], F32, tag=f"lo2_{d}", name=f"lo2_{d}")
            nc.vector.memset(lo2[:], 0.0)
            louts2.append(lo2)
        l2in = keep.tile([128, 4 * 512], BF, tag="l2in")
        # scan scratch tiles (shared by both layers, both dirs)
        Ts = keep.tile([128, 32], F32, tag="T", name="T")
        nc.vector.memset(Ts[:], 0.0)
        npres = keep.tile([128, 16], F32, tag="npre", name="npre")
        As = keep.tile([128, 16], F32, tag="A", name="A")
        fouts = keep.tile([128, 16], F32, tag="fout", name="fout")
        hs = keep.tile([128, 8], BF, tag="h_both", name="h_both")
        nc.vector.memset(hs[:], 0.0)
        dummies = None
        if ndummy > 0:
            dpin = keep.tile([128, 1], BF, tag="dpin", name="dpin")
            nc.vector.memset(dpin[:], 0.0)
            scrap = wps.tile([128, 512], F32, tag="warm", name="warm")
            dummies = (scrap, dpin, ndummy)

        # ================== phase 1: convs ==================
        a1c = ar1.tile([128, X9_LEN + X3_LEN], BF, tag="ar1")
        x9 = a1c[:, 0:X9_LEN]
        x3 = a1c[:, X9_LEN:X9_LEN + X3_LEN]
        a2c = ar2.tile([128, X2_LEN + YPST_LEN], BF, tag="ar2")
        x2 = a2c[:, 0:X2_LEN]
        ypst = a2c[:, X2_LEN:X2_LEN + YPST_LEN]
        nc.vector.memset(a1c[:], 0.0)
        nc.vector.memset(a2c[:], 0.0)

        with tc.tile_pool(name="cw", bufs=1) as cw, \
             tc.tile_pool(name="p1psum", bufs=4, space="PSUM") as pp1, \
             tc.tile_pool(name="p1tmp", bufs=3) as tp1:
            w1s = cw.tile([128, 64], BF)
            nc.sync.dma_start(w1s, w1s_d[:])
            w2s = cw.tile([128, 9 * 128], BF)
            nc.sync.dma_start(w2s[:].rearrange("p (s j) -> p s j", s=9),
                              w2s_d[:].rearrange("s p j -> p s j"))
            w3s = cw.tile([128, 9 * 2 * 128], BF)
            nc.sync.dma_start(
                w3s[:].rearrange("p (s c j) -> p s c j", s=9, c=2),
                w3s_d[:].rearrange("s p c j -> p s c j"))
            cb1 = cw.tile([64, 1], F32)
            nc.sync.dma_start(cb1, cb1_d[:])
            sc1 = cw.tile([64, 1], F32)
            nc.sync.dma_start(sc1, sc1_d[:])
            sh1 = cw.tile([64, 1], F32)
            nc.sync.dma_start(sh1, sh1_d[:])
            cb2 = cw.tile([128, 1], F32)
            nc.sync.dma_start(cb2, cb2_d[:])
            sc2 = cw.tile([128, 1], F32)
            nc.sync.dma_start(sc2, sc2_d[:])
            sh2 = cw.tile([128, 1], F32)
            nc.sync.dma_start(sh2, sh2_d[:])
            cb3 = cw.tile([128, 2], F32)
            nc.sync.dma_start(cb3, cb3_d[:])
            sc3 = cw.tile([128, 2], F32)
            nc.sync.dma_start(sc3, sc3_d[:])
            sh3 = cw.tile([128, 2], F32)
            nc.sync.dma_start(sh3, sh3_d[:])

            for i in range(8):
                t0 = i * 128
                for dh in range(3):
                    for dw in range(3):
                        s = dh * 3 + dw
                        start = (t0 + dh) * 130 + dw
                        nc.sync.dma_start(x9[s:s + 1, 0:132 * 130],
                                          xp_d[ds(start, 132 * 130)][None, :])
                # ---- conv1: 33 chunks of (4 rows x 128 f)
                for c in range(33):
                    psum = pp1.tile([128, 512], F32, tag="cpsum")
                    rhs = x9[:, c * 520:c * 520 + 520].rearrange(
                        "p (r w) -> p r w", w=130)[:, :, 0:128]
                    nc.tensor.matmul(psum[0:64], w1s, rhs, start=True, stop=True)
                    tmp = tp1.tile([64, 512], BF, tag="c1tmp")
                    nc.scalar.activation(tmp, psum[0:64], AF.Relu, bias=cb1)
                    tr = tmp[:].rearrange("q (r f e) -> q r f e", f=64, e=2)
                    pm = tp1.tile([64, 256], BF, tag="c1pm")
                    pmr = pm[:].rearrange("q (r f) -> q r f", f=64)
                    nc.vector.tensor_tensor(pmr, tr[:, :, :, 0], tr[:, :, :, 1],
                                            OP.max)
                    xv = x2[0:64, c * 264:c * 264 + 264].rearrange(
                        "q (r w) -> q r w", w=66)[:, :, 1:65]
                    nc.vector.scalar_tensor_tensor(
                        xv, pmr, sc1, sh1[:, 0:1, None].to_broadcast(pmr.shape),
                        OP.mult, OP.add)
                if i == 0:
                    nc.vector.memset(x2[0:64, 0:132], 0.0)
                if i == 7:
                    nc.vector.memset(x2[0:64, 130 * 66:132 * 66], 0.0)
                # ---- conv2: 17 chunks of (<=8 rows x 64 f)
                for c in range(17):
                    r0 = c * 8
                    rows = min(8, 130 - r0)
                    nfree = rows * 64
                    psum = pp1.tile([128, 512], F32, tag="cpsum")
                    for si in range(9):
                        dh, dw = si // 3, si % 3
                        off = (r0 + dh) * 66 + dw
                        rhs = x2[:, off:off + rows * 66].rearrange(
                            "p (r w) -> p r w", w=66)[:, :, 0:64]
                        nc.tensor.matmul(psum[:, 0:nfree],
                                         w2s[:, si * 128:(si + 1) * 128],
                                         rhs, start=(si == 0), stop=(si == 8))
                    tmp = tp1.tile([128, 512], BF, tag="c2tmp")
                    nc.scalar.activation(tmp[:, 0:nfree], psum[:, 0:nfree],
                                         AF.Relu, bias=cb2)
                    tr = tmp[:, 0:nfree].rearrange("p (r f e) -> p r f e",
                                                   f=32, e=2)
                    pm = tp1.tile([128, 256], BF, tag="c2pm")
                    pmr = pm[:, 0:rows * 32].rearrange("p (r f) -> p r f", f=32)
                    nc.vector.tensor_tensor(pmr, tr[:, :, :, 0], tr[:, :, :, 1],
                                            OP.max)
                    xv = x3[:, r0 * 34:r0 * 34 + rows * 34].rearrange(
                        "p (r w) -> p r w", w=34)[:, :, 1:33]
                    nc.vector.scalar_tensor_tensor(
                        xv, pmr, sc2, sh2[:, 0:1, None].to_broadcast(pmr.shape),
                        OP.mult, OP.add)
                if i == 0:
                    nc.vector.memset(x3[:, 0:34], 0.0)
                if i == 7:
                    nc.vector.memset(x3[:, 129 * 34:130 * 34], 0.0)
                # ---- conv3: 2 co-chunks x 8 chunks of (16 rows x 32 f)
                for ch in range(2):
                    for c in range(8):
                        r0 = c * 16
                        psum = pp1.tile([128, 512], F32, tag="cpsum")
                        for si in range(9):
                            dh, dw = si // 3, si % 3
                            off = (r0 + dh) * 34 + dw
                            rhs = x3[:, off:off + 16 * 34].rearrange(
                                "p (r w) -> p r w", w=34)[:, :, 0:32]
                            nc.tensor.matmul(
                                psum,
                                w3s[:, (si * 2 + ch) * 128:(si * 2 + ch + 1) * 128],
                                rhs, start=(si == 0), stop=(si == 8))
                        tmp = tp1.tile([128, 512], BF, tag="c3tmp")
                        nc.scalar.activation(tmp, psum, AF.Relu,
                                             bias=cb3[:, ch:ch + 1])
                        # nest (f, r) for the permuted-yp write
                        tr = tmp[:].rearrange("p (r f e) -> p f r e", f=16, e=2)
                        pm = tp1.tile([128, 256], BF, tag="c3pm")
                        pmr = pm[:].rearrange("p (f r) -> p f r", r=16)
                        nc.vector.tensor_tensor(pmr, tr[:, :, :, 0],
                                                tr[:, :, :, 1], OP.max)
                        yv = ypst[:].rearrange("p (f c t) -> p f c t",
                                               f=16, c=2)[:, :, ch, r0:r0 + 16]
                        nc.vector.scalar_tensor_tensor(
                            yv, pmr, sc3[:, ch:ch + 1],
                            sh3[:, ch:ch + 1, None].to_broadcast(pmr.shape),
                            OP.mult, OP.add)
                nc.sync.dma_start(yp_d[:, :, ds(t0, 128)],
                                  ypst[:].rearrange("p (k t) -> p k t", k=32))

        # ================== phase 2: gx1 (time-major, bwd reversed) =======
        gxf_t = ar1.tile([128, 16384], BF, tag="ar1")
        gxb_t = ar2.tile([128, 16384], BF, tag="ar2")
        gxf = gxf_t[:].rearrange("p (t m) -> p t m", m=16)
        gxb = gxb_t[:].rearrange("p (t m) -> p t m", m=16)
        gxt = (gxf, gxb)
        with tc.tile_pool(name="ypsb", bufs=1) as ypp, \
             tc.tile_pool(name="wi1sb", bufs=2) as wip, \
             tc.tile_pool(name="p2psum", bufs=4, space="PSUM") as pp2:
            for half in range(2):
                ypsb = ypp.tile([128, 16, 1024], BF, tag="ypsb")
                nc.sync.dma_start(ypsb, yp_d[:, ds(half * 16, 16), :])
                for d in range(2):
                    for mc in range(12):
                        wisb = wip.tile([128, 16 * 128], BF, tag="wi1t")
                        nc.sync.dma_start(
                            wisb[:].rearrange("p (k j) -> p k j", k=16),
                            wi1_d[d, mc, :, ds(half * 16, 16), :])
                        for tch in range(2):
                            psum = pp2.tile([128, 512], F32, tag="gxpsum")
                            for kc in range(16):
                                nc.tensor.matmul(
                                    psum, wisb[:, kc * 128:(kc + 1) * 128],
                                    ypsb[:, kc, ds(tch * 512, 512)],
                                    start=(kc == 0), stop=(kc == 15))
                            if d == 1:
                                gview = gxt[d][:, 1023 - tch * 512::-1, mc][:, 0:512]
                            else:
                                gview = gxt[d][:, tch * 512:(tch + 1) * 512, mc]
                            if half == 0:
                                nc.vector.tensor_scalar_add(
                                    gview, psum, gxb1_sb[d][:, mc:mc + 1])
                            else:
                                nc.vector.tensor_tensor(gview, gview, psum,
                                                        OP.add)
        for d in range(2):
            nc.vector.tensor_copy(
                gxt[d][:, :, 12:16],
                bhn1_sb[d][:, None, :].to_broadcast((128, 1024, 4)))
        if debug_outputs:
            nc.sync.dma_start(gx_dbg[0], gxf[:, :, 0:12])
            nc.sync.dma_start(gx_dbg[1], gxb[:, :, 0:12])

        # ================== phase 3: L1 scans ==================
        with tc.tile_pool(name="l1w", bufs=1) as l1w:
            wh1sb = []
            for d in range(2):
                w = l1w.tile([128, 4 * 12 * 128], BF, tag=f"wh1_{d}")
                nc.sync.dma_start(w, wh1_d[d])
                wh1sb.append(w)
            _scan_loop(nc, tc, ps, nblk1, wh1sb, gxt, bhn1_sb, ident, Ts,
                       npres, As, fouts, hs, louts1, BLK=1024 // nblk1,
                       static=static, dummies=dummies)

        nc.vector.tensor_tensor(l2in,
                                louts1[0][:].rearrange("p k s -> p (k s)"),
                                louts1[1][:].rearrange("p k s -> p (k s)"),
                                OP.add)
        if debug_outputs:
            l2f = keep.tile([128, 4 * 512], F32, tag="l2dbg")
            nc.vector.tensor_copy(l2f, l2in)
            nc.sync.dma_start(l2in_dbg[:], l2f)

        # reset h for layer 2
        nc.vector.memset(hs[:], 0.0)

        # ================== phase 4: gx2 + L2 scans ==================
        gx2_t = ar1.tile([128, 16384], BF, tag="ar1")
        gx2f = gx2_t[:, 0:8192].rearrange("p (t m) -> p t m", m=16)
        gx2b = gx2_t[:, 8192:16384].rearrange("p (t m) -> p t m", m=16)
        gx2t = (gx2f, gx2b)
        w4 = ar2.tile([128, 24576], BF, tag="ar2")
        wi2sb = [w4[:, 12288:18432], w4[:, 18432:24576]]
        wh2sb = []
        for d in range(2):
            w2t = keep.tile([128, 6144], BF, tag=f"wh2_{d}", name=f"wh2_{d}")
            nc.sync.dma_start(w2t, wh2_d[d])
            wh2sb.append(w2t)
            nc.sync.dma_start(wi2sb[d], wi2_d[d])
        with tc.tile_pool(name="p4psum", bufs=4, space="PSUM") as pp4:
            for d in range(2):
                for mc in range(12):
                    psum = pp4.tile([128, 512], F32, tag="gx2psum")
                    for kc in range(4):
                        nc.tensor.matmul(
                            psum,
                            wi2sb[d][:, (mc * 4 + kc) * 128:(mc * 4 + kc + 1) * 128],
                            l2in[:, kc * 512:(kc + 1) * 512],
                            start=(kc == 0), stop=(kc == 3))
                    if d == 1:
                        gview = gx2t[d][:, 511::-1, mc][:, 0:512]
                    else:
                        gview = gx2t[d][:, :, mc]
                    nc.vector.tensor_scalar_add(gview, psum,
                                                gxb2_sb[d][:, mc:mc + 1])

            for d in range(2):
                nc.vector.tensor_copy(
                    gx2t[d][:, :, 12:16],
                    bhn2_sb[d][:, None, :].to_broadcast((128, 512, 4)))
            _scan_loop(nc, tc, ps, nblk2, wh2sb, gx2t, bhn2_sb, ident, Ts,
                       npres, As, fouts, hs, louts2, BLK=512 // nblk2,
                       static=static, dummies=dummies)

            osb = keep.tile([128, 4 * 256], F32, tag="osb")
            nc.vector.tensor_tensor(osb,
                                    louts2[0][:].rearrange("p k s -> p (k s)"),
                                    louts2[1][:].rearrange("p k s -> p (k s)"),
                                    OP.add)
            nc.sync.dma_start(out_d[:],
                              osb[:].rearrange("p (k s) -> p k s", k=4))

    nc.compile()
    return nc


# --------------------------------------------------------------------------
# host-side preprocessing
# --------------------------------------------------------------------------

def _bn(g, be, rm, rv):
    s = np.asarray(g) / np.sqrt(np.asarray(rv) + BN_EPS)
    return (s.astype(np.float32),
            (np.asarray(be) - np.asarray(rm) * s).astype(np.float32))


def _prep_common(inputs):
    d = {}
    cw1 = np.asarray(inputs['cw1'])
    w1s = np.zeros((128, 64), np.float32)
    for dh in range(3):
        for dw in range(3):
            w1s[dh * 3 + dw] = cw1[:, 0, dh, dw]
    d['w1s'] = w1s.astype(BF16)
    w2 = np.asarray(inputs['cw2'])
    w2s = np.zeros((9, 128, 128), np.float32)
    w2s[:, 0:64, :] = w2.transpose(2, 3, 1, 0).reshape(9, 64, 128)
    d['w2s'] = w2s.astype(BF16)
    w3 = np.asarray(inputs['cw3'])
    d['w3s'] = np.ascontiguousarray(
        w3.transpose(2, 3, 1, 0).reshape(9, 128, 2, 128)).astype(BF16)
    sc1, sh1 = _bn(inputs['g1'], inputs['be1'], inputs['rm1'], inputs['rv1'])
    sc2, sh2 = _bn(inputs['g2'], inputs['be2'], inputs['rm2'], inputs['rv2'])
    sc3, sh3 = _bn(inputs['g3'], inputs['be3'], inputs['rm3'], inputs['rv3'])
    d['ident'] = np.eye(128, dtype=np.float32).astype(BF16)
    d['cb1'] = np.asarray(inputs['cb1'], np.float32).reshape(64, 1)
    d['sc1'] = sc1.reshape(64, 1)
    d['sh1'] = sh1.reshape(64, 1)
    d['cb2'] = np.asarray(inputs['cb2'], np.float32).reshape(128, 1)
    d['sc2'] = sc2.reshape(128, 1)
    d['sh2'] = sh2.reshape(128, 1)
    d['cb3'] = np.ascontiguousarray(
        np.asarray(inputs['cb3'], np.float32).reshape(2, 128).T)
    d['sc3'] = np.ascontiguousarray(sc3.reshape(2, 128).T)
    d['sh3'] = np.ascontiguousarray(sh3.reshape(2, 128).T)

    dprime = np.arange(4096)
    perm = (dprime % 256) * 16 + dprime // 256

    wi1 = np.zeros((2, 12, 128, 32, 128), np.float32)
    gxb1 = np.zeros((2, 128, 12), np.float32)
    wh1 = np.zeros((2, 128, 4 * 12 * 128), np.float32)
    bhn1 = np.zeros((2, 128, 4), np.float32)
    wi2 = np.zeros((2, 128, 12 * 4 * 128), np.float32)
    gxb2 = np.zeros((2, 128, 12), np.float32)
    wh2 = np.zeros((2, 128, 4 * 12 * 128), np.float32)
    bhn2 = np.zeros((2, 128, 4), np.float32)
    for di, tag in enumerate('fb'):
        wi = np.asarray(inputs[f'wi{tag}1'])[:, perm]
        wi1[di] = wi.reshape(12, 128, 32, 128).transpose(0, 3, 2, 1)
        bias = np.asarray(inputs[f'bi{tag}1']).copy()
        bh = np.asarray(inputs[f'bh{tag}1'])
        bias[:1024] += bh[:1024]
        gxb1[di] = bias.reshape(12, 128).T
        wh1[di] = np.asarray(inputs[f'wh{tag}1']).reshape(
            12, 128, 4, 128).transpose(3, 2, 0, 1).reshape(128, -1)
        bhn1[di] = bh[1024:].reshape(4, 128).T
        wi2v = np.asarray(inputs[f'wi{tag}2'])
        wi2[di] = wi2v.reshape(12, 128, 4, 128).transpose(
            3, 0, 2, 1).reshape(128, -1)
        bias2 = np.asarray(inputs[f'bi{tag}2']).copy()
        bh2 = np.asarray(inputs[f'bh{tag}2'])
        bias2[:1024] += bh2[:1024]
        gxb2[di] = bias2.reshape(12, 128).T
        wh2[di] = np.asarray(inputs[f'wh{tag}2']).reshape(
            12, 128, 4, 128).transpose(3, 2, 0, 1).reshape(128, -1)
        bhn2[di] = bh2[1024:].reshape(4, 128).T
    d['wi1'] = wi1.astype(BF16)
    d['gxb1'] = gxb1
    d['wh1'] = wh1.astype(BF16)
    d['bhn1'] = bhn1.astype(BF16)
    d['wi2'] = wi2.astype(BF16)
    d['gxb2'] = gxb2
    d['wh2'] = wh2.astype(BF16)
    d['bhn2'] = bhn2.astype(BF16)
    return d


def _prep_sample(x_c):
    xp = np.zeros((1031, 130), np.float32)
    xp[3:1027, 1:129] = x_c
    return {'xp': xp.astype(BF16).reshape(-1)}


def get_nc(debug_outputs=False, nblk1=16, nblk2=8, ndummy=0):
    key = (bool(debug_outputs), nblk1, nblk2, ndummy)
    if key not in _CACHED_NC:
        _CACHED_NC[key] = build_nc(debug_outputs, nblk1, nblk2, ndummy=ndummy)
    return _CACHED_NC[key]


def run(inputs, debug_outputs=False, nblk1=16, nblk2=8, ndummy=NDUMMY, **rkw):
    nc = get_nc(debug_outputs, nblk1, nblk2, ndummy)
    common = _prep_common(inputs)
    x = np.asarray(inputs['x'])
    in_maps = []
    for c in range(8):
        m = dict(common)
        m.update(_prep_sample(x[c, 0]))
        in_maps.append(m)
    return run_bass_kernel_spmd(nc, in_maps, core_ids=list(range(8)), **rkw)


def kernel(**inputs) -> np.ndarray:
    res = run(inputs)
    outs = []
    for c in range(8):
        o = np.asarray(res.results[c]['out'])  # [128, 4, 256]
        outs.append(np.ascontiguousarray(
            o.transpose(2, 1, 0).reshape(256, 512)))
    return np.stack(outs).astype(np.float32)


# revision 20
# speedup vs baseline: 1.1828x; 1.1828x over previous
"""Trainium2 Bass kernel for nn_CategoryBranch2 (3 conv blocks + 2 BiGRU layers).

Distribution: data-parallel over batch B=8 -> one sample per NeuronCore; each
core runs the whole pipeline for its sample.

Device plan per core:
  phase1  conv blocks, T-tiled (8 tiles of 128 steps, halo recompute).
          conv1 via im2col over 9 shift-partitions (single K=128 matmul per
          output chunk); conv2/3 via 9 shift-accumulated matmuls. Epilogues
          fuse bias+relu (ACT), maxpool over f (DVE max), BN affine (DVE).
          conv3 output is written in the permuted layout d' = f*256 + c so the
          GRU input matmul needs no device transpose (wi1 columns are permuted
          on host to match); staged to DRAM as yp[128, 32, 1024].
  phase2  gx1[dir] = wi1' @ y' + (bi + bh_rz), fp32, SBUF-resident,
          TIME-MAJOR [128, 1024, 12] per dir; bwd stored time-reversed.
  phase3  L1 GRU scans (1024 steps). Per step: one 16-col identity-matmul
          preload per dir injects gx_rz/gx_n/bh_n (pre-replicated into the
          gx rows) into PSUM, then 96 bf16 weight matmuls, then ONE lockstep
          tail for both dirs: sigmoid -> q=r*v -> npre=q+gxn -> tanh(n) ->
          f-scan (tensor_tensor_scan FMA: zc*n+z*h via (0,x) interleave) ->
          tanh(h).  zc/a are computed on GPSIMD off the critical path.
  phase4  gx2 + L2 scans (512 steps), same machinery; final sum ->
          out[128, 4, 256] (host reassembles).

PSUM per-step layout (ONE bank tile [128,32] for both dirs, 4 buffers;
base=16d per dir): u 0:8 | gxn 8:12 | v 12:16.  The gx rows are 16 wide:
[rz(8) | n(4) | bhn replica(4)] so a single contiguous identity matmul
preloads everything (start=True on d0 zeroes the bank).
SBUF T [128,32]: r at 8d+2j+1, z->1-z in place at 16+8d+2j+1; even cols
stay 0 (memset once) = multiply-by-zero state resets for the f-scan.

Self-contained: hardcodes all shapes; host does only numpy weight re-layouts.
"""

import numpy as np
import ml_dtypes

import concourse.bacc as bacc
import concourse.bass as bass
import concourse.mybir as mybir
from concourse.tile import TileContext
from concourse.bass import ds
from concourse.bass_utils import run_bass_kernel_spmd

BF16 = ml_dtypes.bfloat16
E4M3 = ml_dtypes.float8_e4m3
F32 = mybir.dt.float32
BF = mybir.dt.bfloat16
FP8 = mybir.dt.float8e4
AF = mybir.ActivationFunctionType
OP = mybir.AluOpType
PE = mybir.EngineType.PE

BN_EPS = 1e-5

X9_LEN = 134 * 130          # 17420
X2_LEN = 132 * 66 + 66      # 8778
X3_LEN = 130 * 34 + 34      # 4454
YPST_LEN = 32 * 128         # 4096

_CACHED_NC = {}
import os as _os
NDUMMY = int(_os.environ.get('KNDUMMY', '0'))


def _scan_superstep(nc, ps, whs, gxvs, bhns, ident, Ts, npres, As, fouts, hs,
                    louts, tgs, s_out, dummies=None):
    """One time step, both dirs lockstep. PE: 1 contiguous preload matmul
    per dir (no h dep) + 96 weight matmuls; then one 6-hop tail shared by
    both dirs: sigma -> q -> npre -> tanh(n) -> f-scan -> tanh(h)."""
    # psum [128,32], base=16d per dir: u 0:8 | gxn 8:12 | v 12:16
    # T [128,32]: r at 8d+2j+1 (for q), z->zc in place at 16+8d+2j+1
    #             (f-scan d0-operand: zeros at evens = state resets)
    # A [128,16]: (n, a) pairs per k=4d+j.  h [128,8]: cols d*4+kc.
    T, npre, A, fout, h = Ts, npres, As, fouts, hs
    psum = ps.tile([128, 32], F32, tag="scan_psum", name="psum")
    # psum layout per dir (base=16d): u 0:8 | gxn 8:12 | v 12:16 -> preloads
    # are two contiguous identity matmuls per dir (no per-column stalls)
    for d in range(2):
        nc.tensor.matmul(psum[:, 16 * d:16 * d + 16], ident,
                         gxvs[d][:, ds(tgs[d], 1), 0:16],
                         start=(d == 0), stop=False, skip_group_check=True)
    for d in range(2):
        for kc in range(4):
            for mc in range(12):
                oc = 16 * d + mc if mc < 8 else 16 * d + 12 + (mc - 8)
                nc.tensor.matmul(
                    psum[:, oc:oc + 1],
                    whs[d][:, (kc * 12 + mc) * 128:(kc * 12 + mc + 1) * 128],
                    h[:, 4 * d + kc:4 * d + kc + 1],
                    start=False, stop=(kc == 3), skip_group_check=True)
    if dummies is not None:
        scrap, dpin, nd = dummies
        for k in range(nd):
            nc.tensor.matmul(scrap[0:1, :], dpin, whs[0][:, 0:512],
                             start=True, stop=True, skip_group_check=True)
    # sigma both dirs: u -> r at T[8d+2j+1], z at T[16+8d+2j+1]
    uin = psum[:].rearrange("p (d q g j) -> p q g d j", d=2, q=2, g=2)[:, 0]
    sout = T[:].rearrange("p (g d j e) -> p g d j e", g=2, d=2, e=2)[:, :, :, :, 1]
    nc.scalar.activation(sout, uin, AF.Sigmoid)
    # q = r*v ; npre = q + gxn  (plain DVE ops on contiguous psum views)
    pv = psum[:].rearrange("p (d c) -> p d c", d=2)
    rview = T[:, 0:16].rearrange("p (d j e) -> p d j e", d=2, e=2)[:, :, :, 1]
    q = npre[:, 0:8].rearrange("p (d j) -> p d j", d=2)
    nc.vector.tensor_tensor(q, rview, pv[:, :, 12:16], OP.mult)
    np2 = npre[:, 8:16].rearrange("p (d j) -> p d j", d=2)
    nc.vector.tensor_tensor(np2, q, pv[:, :, 8:12], OP.add)
    # tanh n -> A evens
    aview = A[:].rearrange("p (k e) -> p k e", e=2)
    nc.scalar.activation(aview[:, :, 0], npre[:, 8:16], AF.Tanh)
    # GPSIMD (off critical path): a = z*h -> A odds; zc = 1-z in place
    zview = T[:, 16:32].rearrange("p (k e) -> p k e", e=2)[:, :, 1]
    nc.gpsimd.tensor_tensor(aview[:, :, 1], zview, h[:], OP.mult)
    nc.gpsimd.tensor_scalar(zview, zview, -1.0, 1.0, OP.mult, OP.add)
    if dummies is not None:
        nc.gpsimd.tensor_copy(dummies[1], T[:, 1:2])
    # f = zc*n + a for both dirs in one scan op
    nc.vector.tensor_tensor_scan(fout[:], T[:, 16:32], A[:], 0.0,
                                 OP.mult, OP.add)
    # h' = tanh(f)
    fodd = fout[:].rearrange("p (k e) -> p k e", e=2)
    nc.scalar.activation(h[:], fodd[:, :, 1], AF.Tanh)
    if s_out is not None:
        for d in range(2):
            nc.vector.tensor_copy(louts[d][:, :, ds(s_out, 1)],
                                  h[:, 4 * d:4 * d + 4, None])


def _scan_blk(nc, ps, base, sbase, whs, gxvs, bhns, ident, Ts, npres, As,
              fouts, hs, louts, BLK, dummies=None):
    for pi in range(BLK // 2):
        for par in range(2):
            t_loc = pi * 2 + par
            tg = base + t_loc
            _scan_superstep(nc, ps, whs, gxvs, bhns, ident, Ts, npres, As,
                            fouts, hs, louts, (tg, tg),
                            (sbase + pi) if par == 0 else None, dummies)


def _scan_loop(nc, tc, ps, nblk, whs, gxvs, bhns, ident, Ts, npres, As, fouts,
               hs, louts, BLK=32, static=False, dummies=None):
    if nblk == 0:
        return
    if static:
        for blki in range(nblk):
            _scan_blk(nc, ps, blki * BLK, blki * (BLK // 2), whs, gxvs, bhns,
                      ident, Ts, npres, As, fouts, hs, louts, BLK, dummies)
        return
    with tc.For_i(0, nblk, 1, hint_engines=(PE,)) as blk:
        _scan_blk(nc, ps, blk * BLK, blk * (BLK // 2), whs, gxvs, bhns,
                  ident, Ts, npres, As, fouts, hs, louts, BLK, dummies)


def build_nc(debug_outputs=False, nblk1=32, nblk2=16, static=False, ndummy=0):
    nc = bacc.Bacc("TRN2", target_bir_lowering=False, debug=False, num_devices=8)

    # ---------------- inputs ----------------
    xp_d = nc.dram_tensor("xp", [1031 * 130], BF, kind="ExternalInput")
    ident_d = nc.dram_tensor("ident", [128, 128], BF, kind="ExternalInput")
    w1s_d = nc.dram_tensor("w1s", [128, 64], BF, kind="ExternalInput")
    w2s_d = nc.dram_tensor("w2s", [9, 128, 128], BF, kind="ExternalInput")
    w3s_d = nc.dram_tensor("w3s", [9, 128, 2, 128], BF, kind="ExternalInput")
    cb1_d = nc.dram_tensor("cb1", [64, 1], F32, kind="ExternalInput")
    sc1_d = nc.dram_tensor("sc1", [64, 1], F32, kind="ExternalInput")
    sh1_d = nc.dram_tensor("sh1", [64, 1], F32, kind="ExternalInput")
    cb2_d = nc.dram_tensor("cb2", [128, 1], F32, kind="ExternalInput")
    sc2_d = nc.dram_tensor("sc2", [128, 1], F32, kind="ExternalInput")
    sh2_d = nc.dram_tensor("sh2", [128, 1], F32, kind="ExternalInput")
    cb3_d = nc.dram_tensor("cb3", [128, 2], F32, kind="ExternalInput")
    sc3_d = nc.dram_tensor("sc3", [128, 2], F32, kind="ExternalInput")
    sh3_d = nc.dram_tensor("sh3", [128, 2], F32, kind="ExternalInput")
    wi1_d = nc.dram_tensor("wi1", [2, 12, 128, 32, 128], BF, kind="ExternalInput")
    gxb1_d = nc.dram_tensor("gxb1", [2, 128, 12], F32, kind="ExternalInput")
    wh1_d = nc.dram_tensor("wh1", [2, 128, 4 * 12 * 128], BF, kind="ExternalInput")
    bhn1_d = nc.dram_tensor("bhn1", [2, 128, 4], BF, kind="ExternalInput")
    wi2_d = nc.dram_tensor("wi2", [2, 128, 12 * 4 * 128], BF, kind="ExternalInput")
    gxb2_d = nc.dram_tensor("gxb2", [2, 128, 12], F32, kind="ExternalInput")
    wh2_d = nc.dram_tensor("wh2", [2, 128, 4 * 12 * 128], BF, kind="ExternalInput")
    bhn2_d = nc.dram_tensor("bhn2", [2, 128, 4], BF, kind="ExternalInput")

    out_d = nc.dram_tensor("out", [128, 4, 256], F32, kind="ExternalOutput")
    yp_kind = "ExternalOutput" if debug_outputs else "Internal"
    yp_d = nc.dram_tensor("yp", [128, 32, 1024], BF, kind=yp_kind)
    if debug_outputs:
        gx_dbg = nc.dram_tensor("gx_dbg", [2, 128, 1024, 12], BF,
                                kind="ExternalOutput")
        l2in_dbg = nc.dram_tensor("l2in_dbg", [128, 4 * 512], F32,
                                  kind="ExternalOutput")

    with TileContext(nc) as tc:
      with tc.tile_pool(name="keep", bufs=1) as keep, \
           tc.tile_pool(name="arena1", bufs=1) as ar1, \
           tc.tile_pool(name="arena2", bufs=1) as ar2, \
           tc.tile_pool(name="scan_ps", bufs=4, space="PSUM") as ps, \
           tc.tile_pool(name="warm_ps", bufs=1, space="PSUM") as wps:
        # ---- long-lived small tiles
        ident = keep.tile([128, 128], BF, tag="ident")
        nc.sync.dma_start(ident, ident_d[:])
        gxb1_sb = []
        bhn1_sb = []
        louts1 = []
        gxb2_sb = []
        bhn2_sb = []
        louts2 = []
        for d in range(2):
            t = keep.tile([128, 12], F32, tag=f"gxb1_{d}")
            nc.sync.dma_start(t, gxb1_d[d])
            gxb1_sb.append(t)
            t = keep.tile([128, 4], BF, tag=f"bhn1_{d}")
            nc.sync.dma_start(t, bhn1_d[d])
            bhn1_sb.append(t)
            lo = keep.tile([128, 4, 512], BF, tag=f"lo1_{d}", name=f"lo1_{d}")
            nc.vector.memset(lo[:], 0.0)
            louts1.append(lo)
            t = keep.tile([128, 12], F32, tag=f"gxb2_{d}")
            nc.sync.dma_start(t, gxb2_d[d])
            gxb2_sb.append(t)
            t = keep.tile([128, 4], BF, tag=f"bhn2_{d}")
            nc.sync.dma_start(t, bhn2_d[d])
            bhn2_sb.append(t)
            lo2 = keep.tile([128, 4, 256], F32, tag=f"lo2_{d}", name=f"lo2_{d}")
            nc.vector.memset(lo2[:], 0.0)
            louts2.append(lo2)
        l2in = keep.tile([128, 4 * 512], BF, tag="l2in")
        # scan scratch tiles (shared by both layers, both dirs)
        Ts = keep.tile([128, 32], F32, tag="T", name="T")
        nc.vector.memset(Ts[:], 0.0)
        npres = keep.tile([128, 16], F32, tag="npre", name="npre")
        As = keep.tile([128, 16], F32, tag="A", name="A")
        fouts = keep.tile([128, 16], F32, tag="fout", name="fout")
        hs = keep.tile([128, 8], BF, tag="h_both", name="h_both")
        nc.vector.memset(hs[:], 0.0)
        dummies = None
        if ndummy > 0:
            dpin = keep.tile([128, 1], BF, tag="dpin", name="dpin")
            nc.vector.memset(dpin[:], 0.0)
            scrap = wps.tile([128, 512], F32, tag="warm", name="warm")
            dummies = (scrap, dpin, ndummy)

        # ================== phase 1: convs ==================
        a1c = ar1.tile([128, X9_LEN + X3_LEN], BF, tag="ar1")
        x9 = a1c[:, 0:X9_LEN]
        x3 = a1c[:, X9_LEN:X9_LEN + X3_LEN]
        a2c = ar2.tile([128, X2_LEN + YPST_LEN], BF, tag="ar2")
        x2 = a2c[:, 0:X2_LEN]
        ypst = a2c[:, X2_LEN:X2_LEN + YPST_LEN]
        nc.vector.memset(a1c[:], 0.0)
        nc.vector.memset(a2c[:], 0.0)

        with tc.tile_pool(name="cw", bufs=1) as cw, \
             tc.tile_pool(name="p1psum", bufs=4, space="PSUM") as pp1, \
             tc.tile_pool(name="p1tmp", bufs=3) as tp1:
            w1s = cw.tile([128, 64], BF)
            nc.sync.dma_start(w1s, w1s_d[:])
            w2s = cw.tile([128, 9 * 128], BF)
            nc.sync.dma_start(w2s[:].rearrange("p (s j) -> p s j", s=9),
                              w2s_d[:].rearrange("s p j -> p s j"))
            w3s = cw.tile([128, 9 * 2 * 128], BF)
            nc.sync.dma_start(
                w3s[:].rearrange("p (s c j) -> p s c j", s=9, c=2),
                w3s_d[:].rearrange("s p c j -> p s c j"))
            cb1 = cw.tile([64, 1], F32)
            nc.sync.dma_start(cb1, cb1_d[:])
            sc1 = cw.tile([64, 1], F32)
            nc.sync.dma_start(sc1, sc1_d[:])
            sh1 = cw.tile([64, 1], F32)
            nc.sync.dma_start(sh1, sh1_d[:])
            cb2 = cw.tile([128, 1], F32)
            nc.sync.dma_start(cb2, cb2_d[:])
            sc2 = cw.tile([128, 1], F32)
            nc.sync.dma_start(sc2, sc2_d[:])
            sh2 = cw.tile([128, 1], F32)
            nc.sync.dma_start(sh2, sh2_d[:])
            cb3 = cw.tile([128, 2], F32)
            nc.sync.dma_start(cb3, cb3_d[:])
            sc3 = cw.tile([128, 2], F32)
            nc.sync.dma_start(sc3, sc3_d[:])
            sh3 = cw.tile([128, 2], F32)
            nc.sync.dma_start(sh3, sh3_d[:])

            for i in range(8):
                t0 = i * 128
                for dh in range(3):
                    for dw in range(3):
                        s = dh * 3 + dw
                        start = (t0 + dh) * 130 + dw
                        nc.sync.dma_start(x9[s:s + 1, 0:132 * 130],
                                          xp_d[ds(start, 132 * 130)][None, :])
                # ---- conv1: 33 chunks of (4 rows x 128 f)
                for c in range(33):
                    psum = pp1.tile([128, 512], F32, tag="cpsum")
                    rhs = x9[:, c * 520:c * 520 + 520].rearrange(
                        "p (r w) -> p r w", w=130)[:, :, 0:128]
                    nc.tensor.matmul(psum[0:64], w1s, rhs, start=True, stop=True)
                    tmp = tp1.tile([64, 512], BF, tag="c1tmp")
                    nc.scalar.activation(tmp, psum[0:64], AF.Relu, bias=cb1)
                    tr = tmp[:].rearrange("q (r f e) -> q r f e", f=64, e=2)
                    pm = tp1.tile([64, 256], BF, tag="c1pm")
                    pmr = pm[:].rearrange("q (r f) -> q r f", f=64)
                    nc.vector.tensor_tensor(pmr, tr[:, :, :, 0], tr[:, :, :, 1],
                                            OP.max)
                    xv = x2[0:64, c * 264:c * 264 + 264].rearrange(
                        "q (r w) -> q r w", w=66)[:, :, 1:65]
                    nc.vector.scalar_tensor_tensor(
                        xv, pmr, sc1, sh1[:, 0:1, None].to_broadcast(pmr.shape),
                        OP.mult, OP.add)
                if i == 0:
                    nc.vector.memset(x2[0:64, 0:132], 0.0)
                if i == 7:
                    nc.vector.memset(x2[0:64, 130 * 66:132 * 66], 0.0)
                # ---- conv2: 17 chunks of (<=8 rows x 64 f)
                for c in range(17):
                    r0 = c * 8
                    rows = min(8, 130 - r0)
                    nfree = rows * 64
                    psum = pp1.tile([128, 512], F32, tag="cpsum")
                    for si in range(9):
                        dh, dw = si // 3, si % 3
                        off = (r0 + dh) * 66 + dw
                        rhs = x2[:, off:off + rows * 66].rearrange(
                            "p (r w) -> p r w", w=66)[:, :, 0:64]
                        nc.tensor.matmul(psum[:, 0:nfree],
                                         w2s[:, si * 128:(si + 1) * 128],
                                         rhs, start=(si == 0), stop=(si == 8))
                    tmp = tp1.tile([128, 512], BF, tag="c2tmp")
                    nc.scalar.activation(tmp[:, 0:nfree], psum[:, 0:nfree],
                                         AF.Relu, bias=cb2)
                    tr = tmp[:, 0:nfree].rearrange("p (r f e) -> p r f e",
                                                   f=32, e=2)
                    pm = tp1.tile([128, 256], BF, tag="c2pm")
                    pmr = pm[:, 0:rows * 32].rearrange("p (r f) -> p r f", f=32)
                    nc.vector.tensor_tensor(pmr, tr[:, :, :, 0], tr[:, :, :, 1],
                                            OP.max)
                    xv = x3[:, r0 * 34:r0 * 34 + rows * 34].rearrange(
                        "p (r w) -> p r w", w=34)[:, :, 1:33]
                    nc.vector.scalar_tensor_tensor(
                        xv, pmr, sc2, sh2[:, 0:1, None].to_broadcast(pmr.shape),
                        OP.mult, OP.add)
                if i == 0:
                    nc.vector.memset(x3[:, 0:34], 0.0)
                if i == 7:
                    nc.vector.memset(x3[:, 129 * 34:130 * 34], 0.0)
                # ---- conv3: 2 co-chunks x 8 chunks of (16 rows x 32 f)
                for ch in range(2):
                    for c in range(8):
                        r0 = c * 16
                        psum = pp1.tile([128, 512], F32, tag="cpsum")
                        for si in range(9):
                            dh, dw = si // 3, si % 3
                            off = (r0 + dh) * 34 + dw
                            rhs = x3[:, off:off + 16 * 34].rearrange(
                                "p (r w) -> p r w", w=34)[:, :, 0:32]
                            nc.tensor.matmul(
                                psum,
                                w3s[:, (si * 2 + ch) * 128:(si * 2 + ch + 1) * 128],
                                rhs, start=(si == 0), stop=(si == 8))
                        tmp = tp1.tile([128, 512], BF, tag="c3tmp")
                        nc.scalar.activation(tmp, psum, AF.Relu,
                                             bias=cb3[:, ch:ch + 1])
                        # nest (f, r) for the permuted-yp write
                        tr = tmp[:].rearrange("p (r f e) -> p f r e", f=16, e=2)
                        pm = tp1.tile([128, 256], BF, tag="c3pm")
                        pmr = pm[:].rearrange("p (f r) -> p f r", r=16)
                        nc.vector.tensor_tensor(pmr, tr[:, :, :, 0],
                                                tr[:, :, :, 1], OP.max)
                        yv = ypst[:].rearrange("p (f c t) -> p f c t",
                                               f=16, c=2)[:, :, ch, r0:r0 + 16]
                        nc.vector.scalar_tensor_tensor(
                            yv, pmr, sc3[:, ch:ch + 1],
                            sh3[:, ch:ch + 1, None].to_broadcast(pmr.shape),
                            OP.mult, OP.add)
                nc.sync.dma_start(yp_d[:, :, ds(t0, 128)],
                                  ypst[:].rearrange("p (k t) -> p k t", k=32))

        # ================== phase 2: gx1 (time-major, bwd reversed) =======
        gx_t = ar1.tile([128, 32768], BF, tag="ar1")
        gxf = gx_t[:, 0:16384].rearrange("p (t m) -> p t m", m=16)
        gxb = gx_t[:, 16384:32768].rearrange("p (t m) -> p t m", m=16)
        gxt = (gxf, gxb)
        with tc.tile_pool(name="ypsb", bufs=1) as ypp, \
             tc.tile_pool(name="wi1sb", bufs=2) as wip, \
             tc.tile_pool(name="p2psum", bufs=4, space="PSUM") as pp2:
            for half in range(2):
                ypsb = ypp.tile([128, 16, 1024], BF, tag="ypsb")
                nc.sync.dma_start(ypsb, yp_d[:, ds(half * 16, 16), :])
                for d in range(2):
                    for mc in range(12):
                        wisb = wip.tile([128, 16 * 128], BF, tag="wi1t")
                        nc.sync.dma_start(
                            wisb[:].rearrange("p (k j) -> p k j", k=16),
                            wi1_d[d, mc, :, ds(half * 16, 16), :])
                        for tch in range(2):
                            psum = pp2.tile([128, 512], F32, tag="gxpsum")
                            for kc in range(16):
                                nc.tensor.matmul(
                                    psum, wisb[:, kc * 128:(kc + 1) * 128],
                                    ypsb[:, kc, ds(tch * 512, 512)],
                                    start=(kc == 0), stop=(kc == 15))
                            if d == 1:
                                gview = gxt[d][:, 1023 - tch * 512::-1, mc][:, 0:512]
                            else:
                                gview = gxt[d][:, tch * 512:(tch + 1) * 512, mc]
                            if half == 0:
                                nc.vector.tensor_scalar_add(
                                    gview, psum, gxb1_sb[d][:, mc:mc + 1])
                            else:
                                nc.vector.tensor_tensor(gview, gview, psum,
                                                        OP.add)
        for d in range(2):
            nc.vector.tensor_copy(
                gxt[d][:, :, 12:16],
                bhn1_sb[d][:, None, :].to_broadcast((128, 1024, 4)))
        if debug_outputs:
            nc.sync.dma_start(gx_dbg[0], gxf[:, :, 0:12])
            nc.sync.dma_start(gx_dbg[1], gxb[:, :, 0:12])

        # ================== phase 3: L1 scans ==================
        with tc.tile_pool(name="l1w", bufs=1) as l1w:
            wh1sb = []
            for d in range(2):
                w = l1w.tile([128, 4 * 12 * 128], BF, tag=f"wh1_{d}")
                nc.sync.dma_start(w, wh1_d[d])
                wh1sb.append(w)
            _scan_loop(nc, tc, ps, nblk1, wh1sb, gxt, bhn1_sb, ident, Ts,
                       npres, As, fouts, hs, louts1, BLK=1024 // nblk1,
                       static=static, dummies=dummies)

        nc.vector.tensor_tensor(l2in,
                                louts1[0][:].rearrange("p k s -> p (k s)"),
                                louts1[1][:].rearrange("p k s -> p (k s)"),
                                OP.add)
        if debug_outputs:
            l2f = keep.tile([128, 4 * 512], F32, tag="l2dbg")
            nc.vector.tensor_copy(l2f, l2in)
            nc.sync.dma_start(l2in_dbg[:], l2f)

        # reset h for layer 2
        nc.vector.memset(hs[:], 0.0)

        # ================== phase 4: gx2 + L2 scans ==================
        gx2_t = ar1.tile([128, 16384], BF, tag="ar1")
        gx2f = gx2_t[:, 0:8192].rearrange("p (t m) -> p t m", m=16)
        gx2b = gx2_t[:, 8192:16384].rearrange("p (t m) -> p t m", m=16)
        gx2t = (gx2f, gx2b)
        w4 = ar2.tile([128, 24576], BF, tag="ar2")
        wi2sb = [w4[:, 12288:18432], w4[:, 18432:24576]]
        wh2sb = []
        for d in range(2):
            w2t = keep.tile([128, 6144], BF, tag=f"wh2_{d}", name=f"wh2_{d}")
            nc.sync.dma_start(w2t, wh2_d[d])
            wh2sb.append(w2t)
            nc.sync.dma_start(wi2sb[d], wi2_d[d])
        with tc.tile_pool(name="p4psum", bufs=4, space="PSUM") as pp4:
            for d in range(2):
                for mc in range(12):
                    psum = pp4.tile([128, 512], F32, tag="gx2psum")
                    for kc in range(4):
                        nc.tensor.matmul(
                            psum,
                            wi2sb[d][:, (mc * 4 + kc) * 128:(mc * 4 + kc + 1) * 128],
                            l2in[:, kc * 512:(kc + 1) * 512],
                            start=(kc == 0), stop=(kc == 3))
                    if d == 1:
                        gview = gx2t[d][:, 511::-1, mc][:, 0:512]
                    else:
                        gview = gx2t[d][:, :, mc]
                    nc.vector.tensor_scalar_add(gview, psum,
                                                gxb2_sb[d][:, mc:mc + 1])

            for d in range(2):
                nc.vector.tensor_copy(
                    gx2t[d][:, :, 12:16],
                    bhn2_sb[d][:, None, :].to_broadcast((128, 512, 4)))
            _scan_loop(nc, tc, ps, nblk2, wh2sb, gx2t, bhn2_sb, ident, Ts,
                       npres, As, fouts, hs, louts2, BLK=512 // nblk2,
                       static=static, dummies=dummies)

            osb = keep.tile([128, 4 * 256], F32, tag="osb")
            nc.vector.tensor_tensor(osb,
                                    louts2[0][:].rearrange("p k s -> p (k s)"),
                                    louts2[1][:].rearrange("p k s -> p (k s)"),
                                    OP.add)
            nc.sync.dma_start(out_d[:],
                              osb[:].rearrange("p (k s) -> p k s", k=4))

    nc.compile()
    return nc


# --------------------------------------------------------------------------
# host-side preprocessing
# --------------------------------------------------------------------------

def _bn(g, be, rm, rv):
    s = np.asarray(g) / np.sqrt(np.asarray(rv) + BN_EPS)
    return (s.astype(np.float32),
            (np.asarray(be) - np.asarray(rm) * s).astype(np.float32))


def _prep_common(inputs):
    d = {}
    cw1 = np.asarray(inputs['cw1'])
    w1s = np.zeros((128, 64), np.float32)
    for dh in range(3):
        for dw in range(3):
            w1s[dh * 3 + dw] = cw1[:, 0, dh, dw]
    d['w1s'] = w1s.astype(BF16)
    w2 = np.asarray(inputs['cw2'])
    w2s = np.zeros((9, 128, 128), np.float32)
    w2s[:, 0:64, :] = w2.transpose(2, 3, 1, 0).reshape(9, 64, 128)
    d['w2s'] = w2s.astype(BF16)
    w3 = np.asarray(inputs['cw3'])
    d['w3s'] = np.ascontiguousarray(
        w3.transpose(2, 3, 1, 0).reshape(9, 128, 2, 128)).astype(BF16)
    sc1, sh1 = _bn(inputs['g1'], inputs['be1'], inputs['rm1'], inputs['rv1'])
    sc2, sh2 = _bn(inputs['g2'], inputs['be2'], inputs['rm2'], inputs['rv2'])
    sc3, sh3 = _bn(inputs['g3'], inputs['be3'], inputs['rm3'], inputs['rv3'])
    d['ident'] = np.eye(128, dtype=np.float32).astype(BF16)
    d['cb1'] = np.asarray(inputs['cb1'], np.float32).reshape(64, 1)
    d['sc1'] = sc1.reshape(64, 1)
    d['sh1'] = sh1.reshape(64, 1)
    d['cb2'] = np.asarray(inputs['cb2'], np.float32).reshape(128, 1)
    d['sc2'] = sc2.reshape(128, 1)
    d['sh2'] = sh2.reshape(128, 1)
    d['cb3'] = np.ascontiguousarray(
        np.asarray(inputs['cb3'], np.float32).reshape(2, 128).T)
    d['sc3'] = np.ascontiguousarray(sc3.reshape(2, 128).T)
    d['sh3'] = np.ascontiguousarray(sh3.reshape(2, 128).T)

    dprime = np.arange(4096)
    perm = (dprime % 256) * 16 + dprime // 256

    wi1 = np.zeros((2, 12, 128, 32, 128), np.float32)
    gxb1 = np.zeros((2, 128, 12), np.float32)
    wh1 = np.zeros((2, 128, 4 * 12 * 128), np.float32)
    bhn1 = np.zeros((2, 128, 4), np.float32)
    wi2 = np.zeros((2, 128, 12 * 4 * 128), np.float32)
    gxb2 = np.zeros((2, 128, 12), np.float32)
    wh2 = np.zeros((2, 128, 4 * 12 * 128), np.float32)
    bhn2 = np.zeros((2, 128, 4), np.float32)
    for di, tag in enumerate('fb'):
        wi = np.asarray(inputs[f'wi{tag}1'])[:, perm]
        wi1[di] = wi.reshape(12, 128, 32, 128).transpose(0, 3, 2, 1)
        bias = np.asarray(inputs[f'bi{tag}1']).copy()
        bh = np.asarray(inputs[f'bh{tag}1'])
        bias[:1024] += bh[:1024]
        gxb1[di] = bias.reshape(12, 128).T
        wh1[di] = np.asarray(inputs[f'wh{tag}1']).reshape(
            12, 128, 4, 128).transpose(3, 2, 0, 1).reshape(128, -1)
        bhn1[di] = bh[1024:].reshape(4, 128).T
        wi2v = np.asarray(inputs[f'wi{tag}2'])
        wi2[di] = wi2v.reshape(12, 128, 4, 128).transpose(
            3, 0, 2, 1).reshape(128, -1)
        bias2 = np.asarray(inputs[f'bi{tag}2']).copy()
        bh2 = np.asarray(inputs[f'bh{tag}2'])
        bias2[:1024] += bh2[:1024]
        gxb2[di] = bias2.reshape(12, 128).T
        wh2[di] = np.asarray(inputs[f'wh{tag}2']).reshape(
            12, 128, 4, 128).transpose(3, 2, 0, 1).reshape(128, -1)
        bhn2[di] = bh2[1024:].reshape(4, 128).T
    d['wi1'] = wi1.astype(BF16)
    d['gxb1'] = gxb1
    d['wh1'] = wh1.astype(BF16)
    d['bhn1'] = bhn1.astype(BF16)
    d['wi2'] = wi2.astype(BF16)
    d['gxb2'] = gxb2
    d['wh2'] = wh2.astype(BF16)
    d['bhn2'] = bhn2.astype(BF16)
    return d


def _prep_sample(x_c):
    xp = np.zeros((1031, 130), np.float32)
    xp[3:1027, 1:129] = x_c
    return {'xp': xp.astype(BF16).reshape(-1)}


def get_nc(debug_outputs=False, nblk1=32, nblk2=16, ndummy=0):
    key = (bool(debug_outputs), nblk1, nblk2, ndummy)
    if key not in _CACHED_NC:
        _CACHED_NC[key] = build_nc(debug_outputs, nblk1, nblk2, ndummy=ndummy)
    return _CACHED_NC[key]


def run(inputs, debug_outputs=False, nblk1=32, nblk2=16, ndummy=NDUMMY, **rkw):
    nc = get_nc(debug_outputs, nblk1, nblk2, ndummy)
    common = _prep_common(inputs)
    x = np.asarray(inputs['x'])
    in_maps = []
    for c in range(8):
        m = dict(common)
        m.update(_prep_sample(x[c, 0]))
        in_maps.append(m)
    return run_bass_kernel_spmd(nc, in_maps, core_ids=list(range(8)), **rkw)


def kernel(**inputs) -> np.ndarray:
    res = run(inputs)
    outs = []
    for c in range(8):
        o = np.asarray(res.results[c]['out'])  # [128, 4, 256]
        outs.append(np.ascontiguousarray(
            o.transpose(2, 1, 0).reshape(256, 512)))
    return np.stack(outs).astype(np.float32)
